# revision 17
# baseline (speedup 1.0000x reference)
"""DirectedGCNConv on 8 Trainium2 NeuronCores.

Math (reference):
    h    = x @ W
    agg  = segment_sum(w_e * h[src_e], tgt_e)
    deg  = segment_sum(w_e, tgt_e);  dinv = rsqrt(max(deg, 1))
    out  = (h + agg) * dinv + bias

Since segment-sum commutes with the feature matmul:
    out[t] = ((x[t] + sum_e w_e x[src_e]) @ W) * dinv[t] + bias
so we aggregate raw x rows first (edge gather + one-hot matmul into PSUM)
and apply the 128x128 weight matmul once per 128-target tile.

Sharding: node-parallel by target. The 100k nodes are packed into
784 tiles of 128 (bin-packed so that, per tile, the incoming-edge count
from each of the 4 source-index banks fits in 2 K-tiles of 128 slots);
each core owns 98 tiles and processes every edge targeting its nodes.
x and W are replicated; no collectives are needed.

The gather uses the Q7 dma_gather instruction (int16 indices, hence the
4 source banks of <=32768 rows). Self-edges with weight 1 fold the
"h" term into the aggregation for free.
"""

import sys

import numpy as np

for _p in ("/opt/trn_rl_repo", "/root/.axon_site/_ro/trn_rl_repo"):
    if _p not in sys.path:
        sys.path.append(_p)

N_NODES = 100000
N_EDGES = 640000
D = 128
N_CORES = 8

P = 128                      # partitions / tile node count
NBANKS = 4
KB = 2                       # K-tiles per (tile, bank)
CAP = KB * P                 # max edge slots per (tile, bank)
K_TOT = NBANKS * KB          # K-tiles per tile (8)
SLOTS_PER_TILE = K_TOT * P   # 1024


def _default_cfg():
    return dict(
        n_nodes=N_NODES,
        n_cores=N_CORES,
        tiles_per_core=98,
        run_tiles=7,          # tiles per gather "run"
        bank_size=25088,      # 4 banks cover 100352 rows; int16-safe
    )


# ----------------------------------------------------------------------------
# Host-side preprocessing
# ----------------------------------------------------------------------------

def _pack_nodes(tgt, w_dummy, cfg, rng_seed=0):
    """Assign each node to a (tile, slot) so that per-(tile, bank)
    edge+self counts are <= CAP.  Returns node_order [n_tiles, P] holding
    node ids (or -1 for phantom slots)."""
    n_nodes = cfg["n_nodes"]
    n_tiles = cfg["n_cores"] * cfg["tiles_per_core"]
    n_slots = n_tiles * P
    bank_size = cfg["bank_size"]

    deg = np.bincount(tgt, minlength=n_nodes)
    src = cfg["_src"]
    bank_of_src = (src // bank_size).astype(np.int64)
    # per-node, per-bank demand (incoming edges by source bank + own self edge)
    d = np.zeros((n_slots, NBANKS), np.int64)
    np.add.at(d, (tgt, bank_of_src), 1)
    node_bank = np.arange(n_nodes) // bank_size
    d[np.arange(n_nodes), node_bank] += 1

    total = d.sum(1)
    order = np.argsort(-total, kind="stable")
    # serpentine init: exactly P nodes (incl. phantoms) per tile
    assign = np.empty(n_slots, np.int64)
    row = np.arange(n_slots) // n_tiles
    pos = np.arange(n_slots) % n_tiles
    assign[order] = np.where(row % 2 == 0, pos, n_tiles - 1 - pos)
    loads = np.zeros((n_tiles, NBANKS), np.int64)
    np.add.at(loads, assign, d)
    counts = np.bincount(assign, minlength=n_tiles)
    assert (counts == P).all()

    # repair: swap nodes out of overloaded (tile, bank)
    members = [list(np.where(assign == t)[0]) for t in range(n_tiles)]
    for _ in range(200):
        over = np.argwhere(loads > CAP)
        if len(over) == 0:
            break
        for t, b in over:
            while loads[t, b] > CAP:
                mem = members[t]
                n_out = mem[int(np.argmax(d[mem, b]))]
                # candidate destination tiles, lightest in bank b
                cand = np.argsort(loads[:, b])
                done = False
                for u in cand[:64]:
                    if u == t:
                        continue
                    memu = members[u]
                    n_in = memu[int(np.argmin(d[memu, b]))]
                    new_u = loads[u] - d[n_in] + d[n_out]
                    new_t = loads[t] - d[n_out] + d[n_in]
                    if (new_u <= CAP).all() and (new_t[b] < loads[t, b]):
                        members[t].remove(n_out)
                        members[u].remove(n_in)
                        members[t].append(n_in)
                        members[u].append(n_out)
                        loads[t] = new_t
                        loads[u] = new_u
                        assign[n_out] = u
                        assign[n_in] = t
                        done = True
                        break
                if not done:
                    raise RuntimeError("packing repair failed")
    assert (loads <= CAP).all(), "node packing failed"

    node_order = np.empty((n_tiles, P), np.int64)
    for t in range(n_tiles):
        node_order[t] = sorted(members[t])
    return node_order


def preprocess(x, edge_index, edge_weight, cfg):
    """Build per-core device arrays.  Returns (per_core_inputs, node_order)."""
    n_nodes = cfg["n_nodes"]
    n_cores = cfg["n_cores"]
    tiles_per_core = cfg["tiles_per_core"]
    run_tiles = cfg["run_tiles"]
    bank_size = cfg["bank_size"]
    n_tiles = n_cores * tiles_per_core
    runs = tiles_per_core // run_tiles
    call_slots = run_tiles * CAP              # idxs per dma_gather call
    k_cols = tiles_per_core * K_TOT           # tgtl/v plane columns per core

    src = np.asarray(edge_index[0], np.int64)
    tgt = np.asarray(edge_index[1], np.int64)
    w = np.asarray(edge_weight, np.float32)
    cfg = dict(cfg)
    cfg["_src"] = src

    node_order = _pack_nodes(tgt, w, cfg)

    # node -> (tile, local slot)
    tile_of = np.empty(n_tiles * P, np.int64)
    local_of = np.empty(n_tiles * P, np.int64)
    flat = node_order.reshape(-1)
    tile_of[flat] = np.arange(n_tiles * P) // P
    local_of[flat] = np.arange(n_tiles * P) % P

    # extended edge list: real edges + self edges (v=1)
    all_src = np.concatenate([src, np.arange(n_nodes)])
    all_tgt = np.concatenate([tgt, np.arange(n_nodes)])
    all_v = np.concatenate([w, np.ones(n_nodes, np.float32)])

    e_tile = tile_of[all_tgt]
    e_bank = all_src // bank_size
    order = np.lexsort((e_bank, e_tile))
    all_src, all_tgt, all_v, e_tile, e_bank = (
        a[order] for a in (all_src, all_tgt, all_v, e_tile, e_bank)
    )
    group_key = e_tile * NBANKS + e_bank
    starts = np.searchsorted(group_key, np.arange(n_tiles * NBANKS))
    ends = np.searchsorted(group_key, np.arange(n_tiles * NBANKS) + 1)

    per_core = []
    for c in range(n_cores):
        idx_plane = np.zeros((runs, NBANKS, call_slots), np.int64)
        tgtl_plane = np.full((P, k_cols), 999.0, np.float32)
        v_plane = np.zeros((P, k_cols), np.float32)
        for tau in range(tiles_per_core):
            gt = c * tiles_per_core + tau
            run, t7 = divmod(tau, run_tiles)
            for b in range(NBANKS):
                g = gt * NBANKS + b
                s0, s1 = starts[g], ends[g]
                n_g = s1 - s0
                assert n_g <= CAP
                sl = slice(s0, s1)
                srcs = all_src[sl] - b * bank_size
                tls = local_of[all_tgt[sl]]
                vs = all_v[sl]
                # slot s -> k-tile q=s//P, partition p=s%P
                s_arr = np.arange(n_g)
                qq = s_arr // P
                pp = s_arr % P
                cols = tau * K_TOT + b * KB + qq
                tgtl_plane[pp, cols] = tls
                v_plane[pp, cols] = vs
                idx_plane[run, b, t7 * CAP + s_arr] = srcs
        # wrap idx: element i of a call -> [i%16, i//16], tiled over 128 parts
        iw = idx_plane.reshape(runs * NBANKS, call_slots // 16, 16)
        iw = np.ascontiguousarray(iw.transpose(2, 0, 1)).reshape(
            16, runs * NBANKS * (call_slots // 16)
        )
        idx_sb = np.tile(iw, (8, 1)).astype(np.int16)
        per_core.append(
            dict(idx_plane=idx_sb, tgtl_plane=tgtl_plane, v_plane=v_plane)
        )
    return per_core, node_order


# ----------------------------------------------------------------------------
# Kernel builder
# ----------------------------------------------------------------------------

def build(cfg, repeat=1):
    import concourse.bass as bass
    import concourse.mybir as mybir
    import concourse.tile as tile
    import concourse.bacc as bacc

    n_cores = cfg["n_cores"]
    tiles_per_core = cfg["tiles_per_core"]
    run_tiles = cfg["run_tiles"]
    bank_size = cfg["bank_size"]
    n_rows = cfg["n_nodes"]
    runs = tiles_per_core // run_tiles
    call_slots = run_tiles * CAP
    k_cols = tiles_per_core * K_TOT
    f32 = mybir.dt.float32

    nc = bacc.Bacc("TRN2", target_bir_lowering=False, debug=False,
                   num_devices=n_cores, num_swdge_queues=4)
    x_d = nc.declare_dram_parameter("x", [n_rows, D], f32, isOutput=False)
    idx_d = nc.declare_dram_parameter(
        "idx_plane", [P, runs * NBANKS * (call_slots // 16)], mybir.dt.int16,
        isOutput=False)
    tgtl_d = nc.declare_dram_parameter("tgtl_plane", [P, k_cols], f32,
                                       isOutput=False)
    v_d = nc.declare_dram_parameter("v_plane", [P, k_cols], f32,
                                    isOutput=False)
    iota_d = nc.declare_dram_parameter("iota", [P, P], f32, isOutput=False)
    iota8_d = nc.declare_dram_parameter("iota8", [P, K_TOT * P], f32,
                                        isOutput=False)
    w_d = nc.declare_dram_parameter("w", [D, D], f32, isOutput=False)
    bias_d = nc.declare_dram_parameter("bias_rep", [P, D], f32, isOutput=False)
    out_d = nc.declare_dram_parameter("out", [tiles_per_core * P, D], f32,
                                      isOutput=True)

    bf16 = mybir.dt.bfloat16
    eq = mybir.AluOpType.is_equal
    mult = mybir.AluOpType.mult
    add = mybir.AluOpType.add
    amax = mybir.AluOpType.max

    with tile.TileContext(nc) as tc:
        with (
            tc.tile_pool(name="const", bufs=1) as cpool,
            tc.tile_pool(name="gath", bufs=2) as gpool,
            tc.tile_pool(name="mt", bufs=4) as mpool,
            tc.tile_pool(name="epi", bufs=3) as epool,
            tc.tile_pool(name="psum", bufs=2, space="PSUM") as ppool,
            tc.tile_pool(name="psum2", bufs=1, space="PSUM") as p2pool,
        ):
            idx_sb = cpool.tile([P, runs * NBANKS * (call_slots // 16)],
                                mybir.dt.int16)
            tgtl_sb = cpool.tile([P, k_cols], f32)
            v_sb = cpool.tile([P, k_cols], f32)
            iota_sb = cpool.tile([P, P], f32)
            iota8_sb = cpool.tile([P, K_TOT * P], f32)
            w_sb = cpool.tile([D, D], f32)
            bias_sb = cpool.tile([P, D], f32)
            ones_sb = cpool.tile([P, 1], f32)
            nc.sync.dma_start(out=idx_sb[:], in_=idx_d[:])
            nc.sync.dma_start(out=tgtl_sb[:], in_=tgtl_d[:])
            nc.sync.dma_start(out=v_sb[:], in_=v_d[:])
            nc.sync.dma_start(out=iota_sb[:], in_=iota_d[:])
            nc.sync.dma_start(out=iota8_sb[:], in_=iota8_d[:])
            nc.sync.dma_start(out=w_sb[:], in_=w_d[:])
            nc.sync.dma_start(out=bias_sb[:], in_=bias_d[:])
            nc.any.memset(ones_sb[:], 1.0)
            # bf16 copies for the 2x-mode one-hot compare (values are small
            # integers, exact in bf16)
            iota8_b = cpool.tile([P, K_TOT * P], bf16)
            tgtl_b = cpool.tile([P, k_cols], bf16)
            nc.vector.tensor_copy(out=iota8_b[:], in_=iota8_sb[:])
            nc.vector.tensor_copy(out=tgtl_b[:], in_=tgtl_sb[:])

            def body(_iv=None):
                for run in range(runs):
                    gts = []
                    for b in range(NBANKS):
                        g = gpool.tile([P, call_slots // P, D], f32,
                                       tag=f"g{b}")
                        call_id = run * NBANKS + b
                        lo = b * bank_size
                        hi = min(lo + bank_size, n_rows)
                        nc.gpsimd.dma_gather(
                            out_ap=g[:],
                            in_ap=x_d[lo:hi, :],
                            idxs_ap=idx_sb[:, call_id * (call_slots // 16):
                                           (call_id + 1) * (call_slots // 16)],
                            num_idxs=call_slots,
                            num_idxs_reg=call_slots,
                            elem_size=D,
                            single_packet=False,
                            queue_num=b,
                        )
                        gts.append(g)
                    for t7 in range(run_tiles):
                        tau = run * run_tiles + t7
                        psumT = ppool.tile([P, P], f32, tag="psumT")
                        pdeg = ppool.tile([1, P], f32, tag="pdeg")
                        # one-hot-with-values M_big[e, k, t] = v * (iota==tgtl)
                        mbig = mpool.tile([P, K_TOT, P], f32, tag="mbig")
                        onehot = mpool.tile([P, K_TOT, P], bf16, tag="onehot")
                        tb = tgtl_b[:, tau * K_TOT:(tau + 1) * K_TOT, None] \
                            .to_broadcast([P, K_TOT, P])
                        vb = v_sb[:, tau * K_TOT:(tau + 1) * K_TOT, None] \
                            .to_broadcast([P, K_TOT, P])
                        nc.vector.tensor_tensor(
                            out=onehot[:],
                            in0=iota8_b[:].rearrange("p (k t) -> p k t",
                                                     k=K_TOT),
                            in1=tb, op=eq)
                        nc.vector.tensor_tensor(
                            out=mbig[:], in0=onehot[:], in1=vb, op=mult)
                        for kk in range(K_TOT):
                            xg = gts[kk // KB][:, t7 * KB + kk % KB, :]
                            nc.tensor.matmul(
                                psumT[:], lhsT=xg, rhs=mbig[:, kk, :],
                                start=(kk == 0), stop=(kk == K_TOT - 1))
                        for kk in range(K_TOT):
                            nc.tensor.matmul(
                                pdeg[:], lhsT=ones_sb[:], rhs=mbig[:, kk, :],
                                start=(kk == 0), stop=(kk == K_TOT - 1))
                        # epilogue
                        s_t = epool.tile([P, P], f32, tag="s_t")
                        nc.scalar.copy(out=s_t[:], in_=psumT[:])
                        deg_sb = epool.tile([1, P], f32, tag="deg")
                        nc.scalar.copy(out=deg_sb[:], in_=pdeg[:])
                        degT = p2pool.tile([P, 1], f32, tag="degT")
                        # [1,128] -> [128,1]: deg_sb.T @ [[1.0]]
                        nc.tensor.matmul(degT[:], lhsT=deg_sb[:],
                                         rhs=ones_sb[0:1, 0:1],
                                         start=True, stop=True)
                        ind = epool.tile([P, 1], f32, tag="ind")
                        nc.vector.tensor_scalar(
                            out=ind[:], in0=degT[:], scalar1=-1.0,
                            scalar2=1.0, op0=add, op1=amax)
                        rec = epool.tile([P, 1], f32, tag="rec")
                        nc.vector.reciprocal(rec[:], ind[:])
                        dinv = epool.tile([P, 1], f32, tag="dinv")
                        nc.scalar.activation(
                            dinv[:], rec[:],
                            func=bass.mybir.ActivationFunctionType.Sqrt)
                        out_ps = p2pool.tile([P, D], f32, tag="out_ps")
                        nc.tensor.matmul(out_ps[:], lhsT=s_t[:], rhs=w_sb[:],
                                         start=True, stop=True)
                        o_sb = epool.tile([P, D], f32, tag="o_sb")
                        nc.vector.tensor_scalar(
                            out=o_sb[:], in0=out_ps[:],
                            scalar1=dinv[:, 0:1], scalar2=None, op0=mult)
                        nc.vector.tensor_tensor(
                            out=o_sb[:], in0=o_sb[:], in1=bias_sb[:], op=add)
                        nc.sync.dma_start(
                            out=out_d[tau * P:(tau + 1) * P, :], in_=o_sb[:])

            if repeat == 1:
                body()
            else:
                with tc.For_i(0, repeat, 1) as iv:
                    body(iv)
    nc.compile()
    return nc


_CACHE = {}


def kernel(x, edge_index, edge_weight, weight, bias):
    from concourse.bass_utils import run_bass_kernel_spmd

    x = np.asarray(x, np.float32)
    edge_index = np.asarray(edge_index, np.int64)
    edge_weight = np.asarray(edge_weight, np.float32)
    weight = np.asarray(weight, np.float32)
    bias = np.asarray(bias, np.float32)

    cfg = _default_cfg()
    per_core, node_order = preprocess(x, edge_index, edge_weight, cfg)

    if "nc" not in _CACHE:
        _CACHE["nc"] = build(cfg)
    nc = _CACHE["nc"]

    iota = np.tile(np.arange(P, dtype=np.float32), (P, 1))
    iota8 = np.tile(iota, (1, K_TOT))
    bias_rep = np.tile(bias[None, :], (P, 1)).astype(np.float32)
    in_maps = []
    for c in range(cfg["n_cores"]):
        in_maps.append(dict(
            x=x,
            idx_plane=per_core[c]["idx_plane"],
            tgtl_plane=per_core[c]["tgtl_plane"],
            v_plane=per_core[c]["v_plane"],
            iota=iota,
            iota8=iota8,
            w=weight,
            bias_rep=bias_rep,
        ))
    res = run_bass_kernel_spmd(nc, in_maps, core_ids=list(range(cfg["n_cores"])))
    rows = np.concatenate([res.results[c]["out"] for c in range(cfg["n_cores"])],
                          axis=0)
    out = np.zeros((N_NODES, D), np.float32)
    flat = node_order.reshape(-1)
    valid = flat < N_NODES
    out[flat[valid]] = rows[valid]
    return out
